# revision 1
# baseline (speedup 1.0000x reference)
"""TRN2 Bass kernel for nn_MultiHeadMusicDecoder (2-layer GRU decoder with
output feedback), data-parallel over batch across 8 NeuronCores.

Strategy per core (B=16 batch rows):
  - The decoder feedback x_{t+1} = Wcat@h1_t + bcat is algebraically fused
    into layer-0's input weights (A = Wih0 @ Wcat), so the sequential loop
    only carries (h0, h1); the three output heads are computed after the
    loop as one batched matmul over all T=512 steps.
  - Layout: gate rows on PSUM partitions, batch on the free dim; weights
    are the stationary matmul operand (fp16, fast weight load).
  - All gate biases are injected into PSUM by one mask-matmul per bank
    (bias rows as a K=6 / K=2 stationary times a 0/1 mask), which also
    opens the accumulation group.
  - Activation chain per layer: sig = sigmoid(P_rz); u = r*Q_nh;
    v = u + P_ni; n = tanh(v); zc = 1-z; h' = z*h_prev + zc*n.
  - h1 history stays in SBUF, is relayouted once, and feeds the batched
    head matmul; outputs DMA straight to HBM in [B, T, V] layout.
"""

import numpy as np
from contextlib import ExitStack

import concourse.bacc as bacc
import concourse.mybir as mybir
import concourse.tile as tile
import concourse.tile_rust as tile_rust
from concourse.bass_utils import run_bass_kernel_spmd

FP32 = mybir.dt.float32
AF = mybir.ActivationFunctionType

H = 256          # d_model / hidden
T = 512          # decoding steps
N_CORES = 8
B = 16           # batch rows per core (128 / 8)
W = 2 * B        # packed free width: 2 K-halves x B
CT = 128 // B    # head-phase steps per chunk
NK = 2           # K tiles per 256-contraction
NM = 6           # M tiles per layer: r0 r1 z0 z1 n0 n1
WCOLS = NM * NK * 128
DT2 = mybir.dt.float16
DT2_NP = np.float16


def _const_layouts():
    lay_h, lay_f = {}, {}
    off = 0
    for n_, w in [("W0i_t0", WCOLS), ("W0h", WCOLS), ("W1i", WCOLS),
                  ("W1h", WCOLS), ("W0i", WCOLS), ("Whead", 2 * H),
                  ("x0T", W),
                  ("bias6_0", 128), ("bias6_0t", 128), ("bias6_1", 128),
                  ("bias2_0", 128), ("bias2_1", 128),
                  ("mask6", 3 * W), ("mask2", W)]:
        lay_h[n_] = (off, w)
        off += w
    lay_f["bcat_b"] = (0, H)
    return lay_h, lay_f


def _arrange_lhsT(Wm):
    Wt = Wm.T
    cols = []
    for m in range(NM):
        for k in range(NK):
            cols.append(Wt[128 * k:128 * (k + 1), 128 * m:128 * (m + 1)])
    return np.ascontiguousarray(np.concatenate(cols, axis=1))


def _lhsT_col(m, k):
    return (m * NK + k) * 128


def _prep_core_inputs(inputs, b_slice):
    f32 = np.float32
    Wih0, Whh0 = np.asarray(inputs["Wih0"], f32), np.asarray(inputs["Whh0"], f32)
    Wih1, Whh1 = np.asarray(inputs["Wih1"], f32), np.asarray(inputs["Whh1"], f32)
    bih0, bhh0 = np.asarray(inputs["bih0"], f32), np.asarray(inputs["bhh0"], f32)
    bih1, bhh1 = np.asarray(inputs["bih1"], f32), np.asarray(inputs["bhh1"], f32)
    Wcat = np.concatenate([inputs["Wn"], inputs["Wd"], inputs["Wg"]], 0).astype(f32)
    bcat = np.concatenate([inputs["bn"], inputs["bd"], inputs["bg"]], 0).astype(f32)

    A = (Wih0.astype(np.float64) @ Wcat.astype(np.float64)).astype(f32)
    b_i0 = (Wih0.astype(np.float64) @ bcat.astype(np.float64)).astype(f32) + bih0

    x0 = np.asarray(inputs["initial_input"], f32)[b_slice, 0, :]
    x0T = np.concatenate([x0.T[:128, :], x0.T[128:, :]], axis=1)

    def bias6(b_i, b_h):
        out = np.zeros((128, 128), f32)
        s = b_i[:2 * H] + b_h[:2 * H]
        for m in range(4):
            out[m, :] = s[m * 128:(m + 1) * 128]
        out[4, :] = b_i[2 * H:2 * H + 128]
        out[5, :] = b_i[2 * H + 128:]
        return out

    def bias2(b_h):
        out = np.zeros((128, 128), f32)
        out[0, :] = b_h[2 * H:2 * H + 128]
        out[1, :] = b_h[2 * H + 128:]
        return out

    mask6 = np.zeros((128, 3 * W), f32)
    for m in range(6):
        mask6[m, m * B:(m + 1) * B] = 1.0
    mask2 = np.zeros((128, W), f32)
    for m in range(2):
        mask2[m, m * B:(m + 1) * B] = 1.0

    parts = {
        "W0i_t0": _arrange_lhsT(Wih0),
        "W0i": _arrange_lhsT(A),
        "W0h": _arrange_lhsT(Whh0),
        "W1i": _arrange_lhsT(Wih1),
        "W1h": _arrange_lhsT(Whh1),
        "Whead": np.ascontiguousarray(
            np.concatenate([Wcat.T[:128, :], Wcat.T[128:, :]], 1)),
        "x0T": np.ascontiguousarray(x0T),
        "bias6_0": bias6(b_i0, bhh0),
        "bias6_0t": bias6(bih0, bhh0),
        "bias6_1": bias6(bih1, bhh1),
        "bias2_0": bias2(bhh0),
        "bias2_1": bias2(bhh1),
        "mask6": mask6,
        "mask2": mask2,
        "bcat_b": np.repeat(bcat[None, :], 128, 0),
    }
    lay_h, lay_f = _const_layouts()
    CHm = np.concatenate([parts[n] for n in lay_h], axis=1).astype(DT2_NP)
    CFm = np.concatenate([parts[n] for n in lay_f], axis=1).astype(f32)
    return {"CONSTH": np.ascontiguousarray(CHm), "CONSTF": np.ascontiguousarray(CFm)}


def _build_nc(psum_bufs=3, act_bufs=3):
    nc = bacc.Bacc(None)
    lay_h, lay_f = _const_layouts()
    wh = sum(w for _, w in lay_h.values())
    wf = sum(w for _, w in lay_f.values())
    CH = nc.declare_dram_parameter("CONSTH", [128, wh], DT2, isOutput=False)
    CF = nc.declare_dram_parameter("CONSTF", [128, wf], FP32, isOutput=False)
    Y_out = nc.declare_dram_parameter("Y", [B, T, H], FP32, isOutput=True)

    with tile.TileContext(nc) as tc, ExitStack() as ctx:
        cp = ctx.enter_context(tc.tile_pool(name="const", bufs=1))
        ch = cp.tile([128, wh], DT2, tag="CH", name="CH")
        cf = cp.tile([128, wf], FP32, tag="CF", name="CF")
        nc.sync.dma_start(ch[:], CH[:])
        nc.sync.dma_start(cf[:], CF[:])

        def sl(name, a, b, p0=0, p1=128):
            base, lay = (ch, lay_h) if name in lay_h else (cf, lay_f)
            off, w = lay[name]
            return base[p0:p1, off + a:off + b]

        h1hist = cp.tile([128, T * W], DT2, tag="h1hist", name="h1hist")
        h0 = [cp.tile([128, W], DT2, tag="h0a", name="h0a"),
              cp.tile([128, W], DT2, tag="h0b", name="h0b")]

        with tc.tile_pool(name="psP", bufs=psum_bufs, space="PSUM") as ppP, \
             tc.tile_pool(name="psQ", bufs=psum_bufs, space="PSUM") as ppQ, \
             tc.tile_pool(name="act", bufs=act_bufs) as ap:

            for t in range(T):
                for layer in (0, 1):
                    if layer == 0:
                        Wi = "W0i_t0" if t == 0 else "W0i"
                        Wh = "W0h"
                        if t == 0:
                            rhs_i = lambda k: sl("x0T", k * B, (k + 1) * B)
                        else:
                            rhs_i = lambda k, _t=t: h1hist[
                                :, (_t - 1) * W + k * B:(_t - 1) * W + (k + 1) * B]
                        hp_tile = h0[(t - 1) % 2] if t > 0 else None
                        rhs_h = (lambda k, _h=hp_tile: _h[:, k * B:(k + 1) * B]) \
                            if t > 0 else None
                        h_prev = hp_tile[:] if t > 0 else None
                        h_dst = h0[t % 2][:]
                        b6 = "bias6_0t" if t == 0 else "bias6_0"
                        b2 = "bias2_0"
                    else:
                        Wi, Wh = "W1i", "W1h"
                        rhs_i = (lambda k, _t=t: h0[_t % 2][:, k * B:(k + 1) * B])
                        rhs_h = (lambda k, _t=t: h1hist[
                            :, (_t - 1) * W + k * B:(_t - 1) * W + (k + 1) * B]) \
                            if t > 0 else None
                        h_prev = h1hist[:, (t - 1) * W:t * W] if t > 0 else None
                        h_dst = h1hist[:, t * W:(t + 1) * W]
                        b6, b2 = "bias6_1", "bias2_1"

                    P = ppP.tile([128, 3 * W], FP32, tag="P", name="P")
                    Q = ppQ.tile([128, W], FP32, tag="Q", name="Q")

                    nc.tensor.matmul(P[:], sl(b6, 0, 128, 0, NM),
                                     sl("mask6", 0, 3 * W, 0, NM),
                                     start=True, stop=False)
                    nc.tensor.matmul(Q[:], sl(b2, 0, 128, 0, 2),
                                     sl("mask2", 0, W, 0, 2),
                                     start=True, stop=(rhs_h is None))

                    if rhs_h is not None:
                        for m in range(4):
                            for k in range(NK):
                                nc.tensor.matmul(
                                    P[:, m * B:(m + 1) * B],
                                    sl(Wh, _lhsT_col(m, k), _lhsT_col(m, k) + 128),
                                    rhs_h(k), start=False, stop=False)
                        for m in range(2):
                            for k in range(NK):
                                nc.tensor.matmul(
                                    Q[:, m * B:(m + 1) * B],
                                    sl(Wh, _lhsT_col(4 + m, k),
                                       _lhsT_col(4 + m, k) + 128),
                                    rhs_h(k), start=False,
                                    stop=(m == 1 and k == NK - 1))
                    for m in range(4):
                        for k in range(NK):
                            nc.tensor.matmul(
                                P[:, m * B:(m + 1) * B],
                                sl(Wi, _lhsT_col(m, k), _lhsT_col(m, k) + 128),
                                rhs_i(k), start=False, stop=False)
                    for m in range(2):
                        for k in range(NK):
                            nc.tensor.matmul(
                                P[:, (4 + m) * B:(5 + m) * B],
                                sl(Wi, _lhsT_col(4 + m, k), _lhsT_col(4 + m, k) + 128),
                                rhs_i(k), start=False,
                                stop=(m == 1 and k == NK - 1))

                    sig = ap.tile([128, 2 * W], FP32, tag="sig", name="sig")
                    nc.scalar.activation(sig[:], P[:, 0:2 * W], AF.Sigmoid)
                    r = sig[:, 0:W]
                    z = sig[:, W:2 * W]
                    u = ap.tile([128, W], FP32, tag="u", name="u")
                    nc.vector.tensor_mul(u[:], r, Q[:])
                    v = ap.tile([128, W], FP32, tag="v", name="v")
                    iv = nc.vector.tensor_add(v[:], u[:], P[:, 2 * W:3 * W])
                    n_t = ap.tile([128, W], FP32, tag="n", name="n")
                    nc.scalar.activation(n_t[:], v[:], AF.Tanh)
                    zc = ap.tile([128, W], FP32, tag="zc", name="zc")
                    izc = nc.vector.tensor_scalar(zc[:], z, -1.0, 1.0,
                                                  mybir.AluOpType.mult,
                                                  mybir.AluOpType.add)
                    tile_rust.add_dep_helper(izc.ins, iv.ins, False, "order")

                    if h_prev is not None:
                        p1 = ap.tile([128, W], FP32, tag="p1", name="p1")
                        ip1 = nc.vector.tensor_mul(p1[:], z, h_prev)
                        tile_rust.add_dep_helper(ip1.ins, iv.ins, False, "order")
                        p2 = ap.tile([128, W], FP32, tag="p2", name="p2")
                        nc.vector.tensor_mul(p2[:], zc[:], n_t[:])
                        nc.vector.tensor_add(h_dst, p1[:], p2[:])
                    else:
                        nc.vector.tensor_mul(h_dst, zc[:], n_t[:])

        # head phase
        hist4 = h1hist[:].rearrange("p (t k b) -> p k t b", k=NK, b=B)
        h1k = [cp.tile([128, T * B], DT2, tag=f"h1k{k}", name=f"h1k{k}")
               for k in range(NK)]
        RC = 8
        for c in range(RC):
            tt = slice(c * T // RC, (c + 1) * T // RC)
            for k in range(NK):
                nc.vector.tensor_copy(
                    h1k[k][:, c * (T // RC) * B:(c + 1) * (T // RC) * B],
                    hist4[:, k, tt, :])
        with tc.tile_pool(name="hps", bufs=4, space="PSUM") as hp, \
             tc.tile_pool(name="hsb", bufs=4) as hb:
            for c in range(T // CT):
                Yp = hp.tile([CT * B, H], FP32, tag="Yp", name="Yp")
                for k in range(NK):
                    nc.tensor.matmul(Yp[:], h1k[k][:, c * 128:(c + 1) * 128],
                                     sl("Whead", k * H, (k + 1) * H),
                                     start=(k == 0), stop=(k == NK - 1))
                Ysb = hb.tile([CT * B, H], FP32, tag="Ysb", name="Ysb")
                nc.vector.tensor_add(Ysb[:], Yp[:], sl("bcat_b", 0, H))
                nc.sync.dma_start(
                    Y_out[:, c * CT:(c + 1) * CT, :].rearrange("b t v -> t b v"),
                    Ysb[:])

    nc.compile()
    return nc


_NC_CACHE = {}


def kernel(**inputs):
    if "nc" not in _NC_CACHE:
        _NC_CACHE["nc"] = _build_nc()
    nc = _NC_CACHE["nc"]

    in_maps = [_prep_core_inputs(inputs, slice(c * B, (c + 1) * B))
               for c in range(N_CORES)]
    res = run_bass_kernel_spmd(nc, in_maps, list(range(N_CORES)))
    Y = np.concatenate([r["Y"] for r in res.results], axis=0)  # [128, T, 256]
    note = np.ascontiguousarray(Y[:, :, 0:128])
    dur = np.ascontiguousarray(Y[:, :, 128:192])
    gap = np.ascontiguousarray(Y[:, :, 192:256])
    return note, dur, gap
